# revision 1
# baseline (speedup 1.0000x reference)
"""Hadamard transform kernel for Trainium2 (8 NeuronCores, SPMD).

Problem: x (8192, 4096) fp32; apply a 128-point Hadamard transform to each
contiguous 128-element group of every row.  Equivalent to
    out = (x.reshape(-1, 128) @ M).reshape(8192, 4096)
where M is the 128x128 butterfly matrix (symmetric, entries +/- 2^-3.5).

Strategy per core (rows sharded 8 ways -> 1024 rows/core):
  - DMA a 128-row tile [128, 4096] to SBUF (rows on partitions).
  - For each 128-col group: PE-transpose the 128x128 block into PSUM
    (contraction dim must live on partitions), copy PSUM->SBUF,
    then matmul(lhsT=block^T, rhs=M) -> PSUM gives the transformed block
    back in natural orientation; copy PSUM->SBUF and DMA the tile out.
"""

import math

import numpy as np

import concourse.bass as bass
import concourse.tile as tile
from concourse import bacc, mybir
from concourse.bass import ts
from concourse.bass_utils import run_bass_kernel_spmd

N_CORES = 8
ROWS, COLS = 8192, 4096
R_CORE = ROWS // N_CORES  # 1024 rows per core
G = 128                   # hadamard group size
NG = COLS // G            # 32 groups per row
NT = R_CORE // 128        # 8 row-tiles per core
QUADS = NG // 4           # 4 groups (one PSUM bank) per quad


def _hadamard_matrix() -> np.ndarray:
    """M = butterfly(I_128): out_row = x_row @ M (M symmetric)."""
    x = np.eye(G, dtype=np.float64)[..., None]
    for _ in range(int(math.log2(G))):
        top = x[..., ::2, :] + x[..., 1::2, :]
        bot = x[..., ::2, :] - x[..., 1::2, :]
        x = np.concatenate((top, bot), axis=-1) * (0.5 ** 0.5)
    return np.ascontiguousarray(x.squeeze(-2).astype(np.float32))


def _build_module():
    nc = bacc.Bacc("TRN2", target_bir_lowering=False, debug=False)
    f32 = mybir.dt.float32
    x_d = nc.dram_tensor("x", [R_CORE, COLS], f32, kind="ExternalInput")
    h_d = nc.dram_tensor("hmat", [G, G], f32, kind="ExternalInput")
    i_d = nc.dram_tensor("ident", [G, G], f32, kind="ExternalInput")
    o_d = nc.dram_tensor("out", [R_CORE, COLS], f32, kind="ExternalOutput")

    with tile.TileContext(nc) as tc:
        with (
            tc.tile_pool(name="const", bufs=1) as cpool,
            tc.tile_pool(name="xin", bufs=6) as xpool,
            tc.tile_pool(name="tt", bufs=8) as tpool,
            tc.tile_pool(name="outb", bufs=6) as opool,
            tc.tile_pool(name="pst", bufs=4, space=bass.MemorySpace.PSUM) as pst,
            tc.tile_pool(name="psm", bufs=4, space=bass.MemorySpace.PSUM) as psm,
        ):
            # PE warmup: dummy transposes with no data deps so the PE's
            # HAM clock-gate opens during the initial DMA wait.
            wsb = cpool.tile([G, G], f32)
            nc.gpsimd.memset(wsb[:], 1.0)
            wp = pst.tile([G, G], f32, tag="pt")
            for _ in range(26):
                nc.tensor.transpose(wp[:], wsb[:], wsb[:])

            hm = cpool.tile([G, G], f32)
            idm = cpool.tile([G, G], f32)
            nc.sync.dma_start(hm[:], h_d[:])
            nc.sync.dma_start(idm[:], i_d[:])

            # chunked 128-row tiles; small leading / trailing chunks
            # shorten pipeline fill and drain.  input DMAs ride the
            # Sync HWDGE ring, output DMAs the Scalar ring: separate
            # sequencers, so a store waiting on compute never blocks
            # the issue of the next load.
            for t in range(NT):
                if t == 0:
                    splits = [1024, 2048, 1024]
                elif t == NT - 1:
                    splits = [1024, 2048, 512, 512]
                else:
                    splits = [2048, 2048]
                c0 = 0
                for cc in splits:
                    xt = xpool.tile([128, cc], f32, tag="xt")
                    nc.sync.dma_start(
                        xt[:], x_d[t * 128:(t + 1) * 128, c0:c0 + cc]
                    )
                    ot = opool.tile([128, cc], f32, tag="ot")
                    for q in range(cc // 512):
                        pt = pst.tile([128, 512], f32, tag="pt")
                        for j in range(4):
                            g = (c0 // G) + q * 4 + j
                            nc.tensor.transpose(
                                pt[:, ts(j, G)],
                                xt[:, ts(q * 4 + j, G)],
                                idm[:],
                            )
                        tt = tpool.tile([128, 512], f32)
                        nc.vector.tensor_copy(tt[:], pt[:])
                        pm = psm.tile([128, 512], f32)
                        for j in range(4):
                            nc.tensor.matmul(
                                pm[:, ts(j, G)], tt[:, ts(j, G)], hm[:]
                            )
                        nc.scalar.copy(ot[:, ts(q, 512)], pm[:])
                    nc.scalar.dma_start(
                        o_d[t * 128:(t + 1) * 128, c0:c0 + cc], ot[:]
                    )
                    c0 += cc

    nc.compile()
    return nc


_NC_CACHE = None


def kernel(x) -> np.ndarray:
    global _NC_CACHE
    x = np.ascontiguousarray(np.asarray(x, dtype=np.float32))
    assert x.shape == (ROWS, COLS)
    if _NC_CACHE is None:
        _NC_CACHE = _build_module()
    nc = _NC_CACHE

    hmat = _hadamard_matrix()
    ident = np.eye(G, dtype=np.float32)
    in_maps = [
        {
            "x": np.ascontiguousarray(x[c * R_CORE:(c + 1) * R_CORE]),
            "hmat": hmat,
            "ident": ident,
        }
        for c in range(N_CORES)
    ]
    res = run_bass_kernel_spmd(nc, in_maps, core_ids=list(range(N_CORES)))
    return np.concatenate([r["out"] for r in res.results], axis=0)



# revision 5
# speedup vs baseline: 1.4590x; 1.4590x over previous
"""Hadamard transform kernel for Trainium2 (8 NeuronCores, SPMD).

Problem: x (8192, 4096) fp32; apply a 128-point Hadamard transform to each
contiguous 128-element group of every row.  Equivalent to
    out = (x.reshape(-1, 128) @ M).reshape(8192, 4096)
where M is the 128x128 butterfly matrix (symmetric, entries +/- 2^-3.5).

bf16 end-to-end (tolerance is 2e-2; bf16 error is ~3e-3):
  - Host pre-scales x by sqrt(2) and casts to bf16; the device matrix is the
    raw +/-1 Hadamard scaled by 2^-4 (exact in bf16), so combined scaling is
    exactly H/sqrt(128).  Halves HBM traffic (DMA floor 94us -> 47us/core)
    and quadruples PE throughput vs fp32.
  - Host upcasts the bf16 result back to fp32.

Strategy per core (rows sharded 8 ways -> 1024 rows/core):
  - DMA a 128-row tile [128, cc] bf16 to SBUF (rows on partitions).
  - For each 128-col group: PE-transpose the 128x128 block into PSUM (bf16),
    copy PSUM->SBUF (vector), then matmul(lhsT=block^T, rhs=M) -> PSUM fp32
    gives the transformed block in natural orientation; copy PSUM->SBUF with
    fp32->bf16 convert (split scalar/vector) and DMA the tile out.
"""

import math

import numpy as np
import ml_dtypes

import concourse.bass as bass
import concourse.tile as tile
from concourse import bacc, mybir
from concourse.bass import ts
from concourse.bass_utils import run_bass_kernel_spmd

N_CORES = 8
ROWS, COLS = 8192, 4096
R_CORE = ROWS // N_CORES  # 1024 rows per core
G = 128                   # hadamard group size
NG = COLS // G            # 32 groups per row
NT = R_CORE // 128        # 8 row-tiles per core

BF16 = ml_dtypes.bfloat16


def _hadamard_raw() -> np.ndarray:
    """Raw +/-1 Sylvester Hadamard matrix of order 128 (symmetric)."""
    h = np.array([[1.0]], dtype=np.float64)
    for _ in range(int(math.log2(G))):
        h = np.block([[h, h], [h, -h]])
    return h


def _build_module():
    nc = bacc.Bacc("TRN2", target_bir_lowering=False, debug=False)
    bf16 = mybir.dt.bfloat16
    f32 = mybir.dt.float32
    x_d = nc.dram_tensor("x", [R_CORE, COLS], bf16, kind="ExternalInput")
    h_d = nc.dram_tensor("hmat", [G, G], bf16, kind="ExternalInput")
    i_d = nc.dram_tensor("ident", [G, G], bf16, kind="ExternalInput")
    o_d = nc.dram_tensor("out", [R_CORE, COLS], bf16, kind="ExternalOutput")

    with tile.TileContext(nc) as tc:
        with (
            tc.tile_pool(name="const", bufs=1) as cpool,
            tc.tile_pool(name="xin", bufs=6) as xpool,
            tc.tile_pool(name="tt", bufs=8) as tpool,
            tc.tile_pool(name="outb", bufs=6) as opool,
            tc.tile_pool(name="pst", bufs=3, space=bass.MemorySpace.PSUM) as pst,
            tc.tile_pool(name="psm", bufs=4, space=bass.MemorySpace.PSUM) as psm,
        ):
            # PE warmup: dummy transposes with no data deps so the PE's
            # HAM clock-gate opens during the initial DMA wait.
            wsb = cpool.tile([G, G], bf16)
            nc.gpsimd.memset(wsb[:], 1.0)
            # padded to a full 2 KiB PSUM bank: a half-bank bf16 tile could
            # share a bank with a neighbouring buf, and PE-write + engine-read
            # of the same bank is a fatal HW error.
            wp = pst.tile([G, G], bf16, tag="pt", padded_shape=[128, 1024])
            for _ in range(26):
                nc.tensor.transpose(wp[:, :G], wsb[:], wsb[:])

            hm = cpool.tile([G, G], bf16)
            idm = cpool.tile([G, G], bf16)
            nc.sync.dma_start(hm[:], h_d[:])
            nc.sync.dma_start(idm[:], i_d[:])

            # chunked 128-row tiles; small leading / trailing chunks
            # shorten pipeline fill and drain.  input DMAs ride the
            # Sync HWDGE ring, output DMAs the Scalar ring: separate
            # sequencers, so a store waiting on compute never blocks
            # the issue of the next load.
            for t in range(NT):
                if t == 0:
                    splits = [1024, 2048, 1024]
                elif t == NT - 1:
                    splits = [1024, 2048, 512, 512]
                else:
                    splits = [2048, 2048]
                c0 = 0
                for cc in splits:
                    xt = xpool.tile([128, cc], bf16, tag="xt")
                    nc.sync.dma_start(
                        xt[:], x_d[t * 128:(t + 1) * 128, c0:c0 + cc]
                    )
                    ot = opool.tile([128, cc], bf16, tag="ot")
                    for q in range(cc // 512):
                        pt = pst.tile(
                            [128, 512], bf16, tag="pt",
                            padded_shape=[128, 1024],
                        )
                        for j in range(4):
                            nc.tensor.transpose(
                                pt[:, ts(j, G)],
                                xt[:, ts(q * 4 + j, G)],
                                idm[:],
                            )
                        tt = tpool.tile([128, 512], bf16)
                        nc.vector.tensor_copy(tt[:], pt[:, :512])
                        pm = psm.tile([128, 512], f32)
                        for j in range(4):
                            nc.tensor.matmul(
                                pm[:, ts(j, G)], tt[:, ts(j, G)], hm[:]
                            )
                        # whole-tile evacuation (reads depend on all four
                        # matmuls -> no same-bank read-while-PE-writes),
                        # alternating engines to balance the load.
                        if q % 2 == 0:
                            nc.scalar.copy(ot[:, ts(q, 512)], pm[:])
                        else:
                            nc.vector.tensor_copy(ot[:, ts(q, 512)], pm[:])
                    nc.scalar.dma_start(
                        o_d[t * 128:(t + 1) * 128, c0:c0 + cc], ot[:]
                    )
                    c0 += cc

    nc.compile()
    return nc


_NC_CACHE = None


def _get_nc():
    global _NC_CACHE
    if _NC_CACHE is None:
        _NC_CACHE = _build_module()
    return _NC_CACHE


def _in_maps(x: np.ndarray) -> list:
    """Shard + bf16-encode the full fp32 input for the 8 cores."""
    xs = (np.asarray(x, dtype=np.float32) * np.float32(math.sqrt(2.0)))
    xb = xs.astype(BF16)
    hmat = (_hadamard_raw() * 0.0625).astype(BF16)
    ident = np.eye(G, dtype=np.float32).astype(BF16)
    return [
        {
            "x": np.ascontiguousarray(xb[c * R_CORE:(c + 1) * R_CORE]),
            "hmat": hmat,
            "ident": ident,
        }
        for c in range(N_CORES)
    ]


def kernel(x) -> np.ndarray:
    assert x.shape == (ROWS, COLS)
    nc = _get_nc()
    res = run_bass_kernel_spmd(nc, _in_maps(x), core_ids=list(range(N_CORES)))
    out = np.concatenate([r["out"] for r in res.results], axis=0)
    return out.astype(np.float32)


# revision 6
# speedup vs baseline: 1.7027x; 1.1670x over previous
"""Hadamard transform kernel for Trainium2 (8 NeuronCores, SPMD).

Problem: x (8192, 4096) fp32; apply a 128-point Hadamard transform to each
contiguous 128-element group of every row.  Equivalent to
    out = (x.reshape(-1, 128) @ M).reshape(8192, 4096)
where M is the 128x128 butterfly matrix (symmetric, entries +/- 2^-3.5).

bf16 end-to-end (tolerance is 2e-2; bf16 error is ~3e-3):
  - Host pre-scales x by sqrt(2) and casts to bf16; the device matrix is the
    raw +/-1 Hadamard scaled by 2^-4 (exact in bf16), so combined scaling is
    exactly H/sqrt(128).  Halves HBM traffic (DMA floor 94us -> 47us/core)
    and quadruples PE throughput vs fp32.
  - Host upcasts the bf16 result back to fp32.

Layout: each core's 1024x4096 shard is viewed as [512, 8192] (row pairs).
Rows are independent, so any row->partition assignment works; this one gives
16 KiB contiguous per-partition DMA lines (vs 4 KiB) and 2 MiB whole-tile
transfers, which is what keeps the DMA near the HBM roofline in bf16.

Per [128, 8192] tile: 64 column-blocks of 128.  PE-transpose each block into
PSUM (bf16), copy PSUM->SBUF (vector), matmul(lhsT=block^T, rhs=M) -> PSUM
fp32 in natural orientation, then evacuate PSUM->SBUF with fp32->bf16
convert (3/4 scalar, 1/4 vector) and DMA the tile out.
"""

import math

import numpy as np
import ml_dtypes

import concourse.bass as bass
import concourse.tile as tile
from concourse import bacc, mybir
from concourse.bass import ts
from concourse.bass_utils import run_bass_kernel_spmd

N_CORES = 8
ROWS, COLS = 8192, 4096
R_CORE = ROWS // N_CORES  # 1024 rows per core
G = 128                   # hadamard group size

# device-side view: row pairs -> 16 KiB per-partition DMA lines
VR, VC = R_CORE // 2, COLS * 2   # [512, 8192]
NT = VR // 128                   # 4 tiles per core

BF16 = ml_dtypes.bfloat16


def _hadamard_raw() -> np.ndarray:
    """Raw +/-1 Sylvester Hadamard matrix of order 128 (symmetric)."""
    h = np.array([[1.0]], dtype=np.float64)
    for _ in range(int(math.log2(G))):
        h = np.block([[h, h], [h, -h]])
    return h


def _build_module():
    nc = bacc.Bacc("TRN2", target_bir_lowering=False, debug=False)
    bf16 = mybir.dt.bfloat16
    f32 = mybir.dt.float32
    x_d = nc.dram_tensor("x", [VR, VC], bf16, kind="ExternalInput")
    h_d = nc.dram_tensor("hmat", [G, G], bf16, kind="ExternalInput")
    i_d = nc.dram_tensor("ident", [G, G], bf16, kind="ExternalInput")
    o_d = nc.dram_tensor("out", [VR, VC], bf16, kind="ExternalOutput")

    with tile.TileContext(nc) as tc:
        with (
            tc.tile_pool(name="const", bufs=1) as cpool,
            tc.tile_pool(name="xin", bufs=3) as xpool,
            tc.tile_pool(name="tt", bufs=8) as tpool,
            tc.tile_pool(name="outb", bufs=3) as opool,
            tc.tile_pool(name="pst", bufs=3, space=bass.MemorySpace.PSUM) as pst,
            tc.tile_pool(name="psm", bufs=4, space=bass.MemorySpace.PSUM) as psm,
        ):
            # PE warmup: dummy transposes with no data deps so the PE's
            # HAM clock-gate opens during the initial DMA wait.
            wsb = cpool.tile([G, G], bf16)
            nc.gpsimd.memset(wsb[:], 1.0)
            # padded to a full 2 KiB PSUM bank: a half-bank bf16 tile could
            # share a bank with a neighbouring buf, and PE-write +
            # engine-read of the same bank is a fatal HW error.
            wp = pst.tile([G, G], bf16, tag="pt", padded_shape=[128, 1024])
            for _ in range(26):
                nc.tensor.transpose(wp[:, :G], wsb[:], wsb[:])

            hm = cpool.tile([G, G], bf16)
            idm = cpool.tile([G, G], bf16)
            nc.sync.dma_start(hm[:], h_d[:])
            nc.sync.dma_start(idm[:], i_d[:])

            # whole-tile 2 MiB DMAs keep descriptor/fixed overheads small;
            # the first/last tiles are split in half to shorten pipeline
            # fill and drain.  input DMAs ride the Sync HWDGE ring, output
            # DMAs the Scalar ring.
            for t in range(NT):
                if t == 0 or t == NT - 1:
                    splits = [VC // 2, VC // 2]
                else:
                    splits = [VC]
                c0 = 0
                for cc in splits:
                    xt = xpool.tile([128, cc], bf16, tag="xt")
                    nc.sync.dma_start(
                        xt[:], x_d[t * 128:(t + 1) * 128, c0:c0 + cc]
                    )
                    ot = opool.tile([128, cc], bf16, tag="ot")
                    for q in range(cc // 512):
                        pt = pst.tile(
                            [128, 512], bf16, tag="pt",
                            padded_shape=[128, 1024],
                        )
                        for j in range(4):
                            nc.tensor.transpose(
                                pt[:, ts(j, G)],
                                xt[:, ts(q * 4 + j, G)],
                                idm[:],
                            )
                        tt = tpool.tile([128, 512], bf16)
                        nc.vector.tensor_copy(tt[:], pt[:, :512])
                        pm = psm.tile([128, 512], f32)
                        for j in range(4):
                            nc.tensor.matmul(
                                pm[:, ts(j, G)], tt[:, ts(j, G)], hm[:]
                            )
                        # whole-tile evacuation (reads depend on all four
                        # matmuls -> no same-bank read-while-PE-writes);
                        # scalar takes 3/4, vector 1/4 to balance load.
                        if q % 4 == 3:
                            nc.vector.tensor_copy(ot[:, ts(q, 512)], pm[:])
                        else:
                            nc.scalar.copy(ot[:, ts(q, 512)], pm[:])
                    nc.scalar.dma_start(
                        o_d[t * 128:(t + 1) * 128, c0:c0 + cc], ot[:]
                    )
                    c0 += cc

    nc.compile()
    return nc


_NC_CACHE = None


def _get_nc():
    global _NC_CACHE
    if _NC_CACHE is None:
        _NC_CACHE = _build_module()
    return _NC_CACHE


def _in_maps(x: np.ndarray) -> list:
    """Shard + bf16-encode the full fp32 input for the 8 cores."""
    xs = np.asarray(x, dtype=np.float32) * np.float32(math.sqrt(2.0))
    xb = xs.astype(BF16)
    hmat = (_hadamard_raw() * 0.0625).astype(BF16)
    ident = np.eye(G, dtype=np.float32).astype(BF16)
    return [
        {
            "x": np.ascontiguousarray(
                xb[c * R_CORE:(c + 1) * R_CORE].reshape(VR, VC)
            ),
            "hmat": hmat,
            "ident": ident,
        }
        for c in range(N_CORES)
    ]


def kernel(x) -> np.ndarray:
    assert x.shape == (ROWS, COLS)
    nc = _get_nc()
    res = run_bass_kernel_spmd(nc, _in_maps(x), core_ids=list(range(N_CORES)))
    out = np.concatenate(
        [r["out"].reshape(R_CORE, COLS) for r in res.results], axis=0
    )
    return out.astype(np.float32)
